# revision 16
# baseline (speedup 1.0000x reference)
"""TRN2 Bass/Tile kernel for nn_EngramUpsampler (dense_mlp), v2.

Reference computation:
    x   = repeat_interleave(engrams, 32, axis=2) + pos_emb   # (B, NW, 512, 1024)
    h   = gelu(x @ w1.T + b1, exact)
    x   = x + h @ w2.T + b2
    out = LayerNorm(x) * gamma + beta

Distribution: data-parallel over the B*NW = 64 (batch, window) slices,
8 windows per NeuronCore; weights/pos_emb replicated.

v2 design (vs baseline 206.7us):
  * First matmul factors through the repeat (21x FLOP cut):
        x @ w1.T = repeat(eng @ w1.T) + pos @ w1.T
    Both projections are computed directly in [e, token] orientation from
    HOST-transposed inputs (ebT = [engT | posT]), eliminating all 48 PE
    transposes and their DVE copies of the baseline.
  * h = gelu(...) is emitted by ScalarE directly as fp8e4 into DoubleRow
    pair layout [128, 8ec, 512w]; the second matmul h @ w2.T runs as
    fp8 DoubleRow (2 contraction chunks per instruction, ~1.4-1.8x PE).
    w2.T is pre-scaled by 256 (host) to dodge fp8 subnormals; the residual
    matmul inputs (eng, pos) are also uploaded x256 in bf16, so PSUM
    uniformly holds 256*x.  LayerNorm is scale-invariant: no descale pass,
    only the rstd constants fold in 2^-8 and eps uses var' = var*2^-16.
  * Residual x0 = repeat(eng) + pos joins the same PSUM accumulation via
    R/I matmuls (TensorE), b2 via a ones-row matmul when nonzero.
  * LayerNorm: bn_stats/bn_aggr on DVE; rsqrt via 3 Newton iterations on
    DVE (initial guess 1/(0.5+0.5v) is ~1% accurate for v near 1);
    normalization via ACT Identity(x*rstd - mu*rstd) on 2 tiles and
    GpSimd tensor_scalar on 2, balancing engine load under the PE time.
"""

import numpy as np
import ml_dtypes

import concourse.bass as bass
import concourse.tile as tile
from concourse import bacc, mybir
from concourse.bass_utils import run_bass_kernel_spmd

FP32 = mybir.dt.float32
BF16 = mybir.dt.bfloat16
FP8 = mybir.dt.float8e4
AF = mybir.ActivationFunctionType
OP = mybir.AluOpType
DR = mybir.MatmulPerfMode.DoubleRow

B, NW, K, D, W = 4, 16, 16, 1024, 512
REP = W // K          # 32
N_CORES = 8
WPC = (B * NW) // N_CORES   # 8 windows per core
TOK = WPC * W               # 4096 tokens per core
DC = D // 128               # 8 chunks of the contraction dim
LN_EPS = 1e-5
NEWTON_ITERS = 2
SCALE = 256.0               # x256 on w2/eng/pos; LN folds it back out

_PROGRAM_CACHE: dict = {}


def _build_program(has_b1: bool, has_b2: bool, has_gb: bool):
    """Emit + compile the per-core SPMD program. Same program runs on all 8
    cores; only the DRAM input contents differ."""
    from contextlib import ExitStack

    nc = bacc.Bacc("TRN2", target_bir_lowering=False, debug=False)

    # ebT[d, 0:128] = eng.T chunk, ebT[d, 128:640] = pos.T chunk (bf16)
    ebt_d = nc.dram_tensor("ebt", [D, 640], BF16, kind="ExternalInput").ap()
    w1t_d = nc.dram_tensor("w1t", [D, D], BF16, kind="ExternalInput").ap()
    # w2p[ki, ec, n] = w2.T[ec*128+ki, n] * 256 in fp8e4 (DoubleRow pairs)
    w2p_d = nc.dram_tensor("w2p", [128, DC * D], FP8, kind="ExternalInput").ap()
    # selt[j, c] = 1 iff c//32 == j: window n / token-tile g residual selector
    # is the slice selt[:, 512n + 128g : ...], picking engram row 16n + t//32.
    selt_d = nc.dram_tensor("selt", [128, TOK], BF16, kind="ExternalInput").ap()
    # eng256[16n + k, :] = eng[n, k, :] * 256 -- all 8 windows in one tile
    eng_d = nc.dram_tensor("eng", [128, D], BF16, kind="ExternalInput").ap()
    post_d = nc.dram_tensor("post", [W, D], BF16, kind="ExternalInput").ap()  # x256
    idn_d = nc.dram_tensor("idn", [128, 128], BF16, kind="ExternalInput").ap()
    if has_b1:
        b1c_d = nc.dram_tensor("b1c", [128, DC], FP32, kind="ExternalInput").ap()
    if has_b2:
        b2r_d = nc.dram_tensor("b2r", [1, D], BF16, kind="ExternalInput").ap()  # x256
    if has_gb:
        gam_d = nc.dram_tensor("gam", [1, D], FP32, kind="ExternalInput").ap()
        bet_d = nc.dram_tensor("bet", [1, D], FP32, kind="ExternalInput").ap()
    out_d = nc.dram_tensor("out", [TOK, D], FP32, kind="ExternalOutput").ap()

    with tile.TileContext(nc) as tc, ExitStack() as ctx:
        consts = ctx.enter_context(tc.tile_pool(name="consts", bufs=1))

        def ctile(shape, dtype, tag):
            return consts.tile(shape, dtype, tag=tag, name=tag)

        # ---- persistent SBUF tensors -------------------------------------
        w1t_sb = [ctile([128, D], BF16, f"w1t{i}") for i in range(DC)]
        ebt_sb = [ctile([128, 640], BF16, f"ebt{i}") for i in range(DC)]
        w2p_sb = ctile([128, DC, D], FP8, "w2p")
        selt_sb = ctile([128, TOK], BF16, "selt")
        id_sb = ctile([128, 128], BF16, "id_sb")
        eng_sb = ctile([128, D], BF16, "eng")
        pos_sb = [ctile([128, D], BF16, f"pos{i}") for i in range(4)]
        epjT = ctile([128, D], BF16, "epjT")               # [e, tok] per ec block
        ppj = ctile([128, DC, W], BF16, "ppj")             # [e, ec, w]
        if has_b1:
            b1c_sb = ctile([128, DC], FP32, "b1c")
        if has_b2:
            ones_sb = ctile([1, 128], BF16, "ones")
            b2_sb = ctile([1, D], BF16, "b2")
        if has_gb:
            gam_row = ctile([1, D], FP32, "gamr")
            bet_row = ctile([1, D], FP32, "betr")
            gam_sb = ctile([128, D], FP32, "gam")
            bet_sb = ctile([128, D], FP32, "bet")

        # ---- loads. Head is DMA-transfer-bound: split the setup-critical
        # w1t+ebt across BOTH hwdge queues (sync, scalar), and push the
        # main-loop-only tensors to the gpsimd SWDGE queue. ---------------
        for i in range(DC):
            if i < DC // 2:
                nc.sync.dma_start(w1t_sb[i][:], w1t_d[i * 128:(i + 1) * 128, :])
                nc.scalar.dma_start(ebt_sb[i][:], ebt_d[i * 128:(i + 1) * 128, :])
            else:
                nc.scalar.dma_start(w1t_sb[i][:], w1t_d[i * 128:(i + 1) * 128, :])
                nc.sync.dma_start(ebt_sb[i][:], ebt_d[i * 128:(i + 1) * 128, :])
        nc.sync.dma_start(
            w2p_sb[:].rearrange("p a b -> p (a b)"), w2p_d[:])
        nc.sync.dma_start(id_sb[:], idn_d[:])
        nc.gpsimd.dma_start(eng_sb[:], eng_d[:])
        for i in range(4):
            nc.gpsimd.dma_start(pos_sb[i][:], post_d[i * 128:(i + 1) * 128, :])
        nc.gpsimd.dma_start(selt_sb[:], selt_d[:])
        if has_b1:
            nc.sync.dma_start(b1c_sb[:], b1c_d[:])
        if has_b2:
            nc.gpsimd.memset(ones_sb[:], 1.0)
            nc.sync.dma_start(b2_sb[:], b2r_d[:])
        if has_gb:
            nc.sync.dma_start(gam_row[:], gam_d[:])
            nc.sync.dma_start(bet_row[:], bet_d[:])
            nc.gpsimd.partition_broadcast(gam_sb[:], gam_row[:])
            nc.gpsimd.partition_broadcast(bet_sb[:], bet_row[:])

        # ---- setup: factored first matmul in [e, token] orientation ------
        # epjT[e128, t] = sum_d w1t[d, e] engT[d, t];  ppj[e128, w] likewise
        with tc.tile_pool(name="spsum", bufs=2, space="PSUM") as spsum:
            for ec in range(DC):
                pje = spsum.tile([128, 128], FP32, tag="pje", name="pje")
                pjp = spsum.tile([128, W], FP32, tag="pjp", name="pjp")
                esl = slice(ec * 128, (ec + 1) * 128)
                for dc in range(DC):
                    nc.tensor.matmul(
                        pje[:],
                        lhsT=w1t_sb[dc][:, esl],
                        rhs=ebt_sb[dc][:, 0:128],
                        start=(dc == 0),
                        stop=(dc == DC - 1),
                    )
                for dc in range(DC):
                    nc.tensor.matmul(
                        pjp[:],
                        lhsT=w1t_sb[dc][:, esl],
                        rhs=ebt_sb[dc][:, 128:640],
                        start=(dc == 0),
                        stop=(dc == DC - 1),
                    )
                nc.vector.tensor_copy(epjT[:, esl], pje[:])
                nc.vector.tensor_copy(ppj[:, ec, :], pjp[:])

        # ---- main loop ---------------------------------------------------
        pre_pool = ctx.enter_context(tc.tile_pool(name="pre", bufs=3))
        h_pool = ctx.enter_context(tc.tile_pool(name="h", bufs=3))
        x_pool = ctx.enter_context(tc.tile_pool(name="xps", bufs=3, space="PSUM"))
        xs_pool = ctx.enter_context(tc.tile_pool(name="xs", bufs=10))
        st_pool = ctx.enter_context(tc.tile_pool(name="st", bufs=4))
        nt_pool = ctx.enter_context(tc.tile_pool(name="nt", bufs=2))
        out_pool = ctx.enter_context(tc.tile_pool(name="ot", bufs=4))

        # LN stats/normalize are batched over PAIRS of windows: one Newton
        # chain of [128, 8] per pair halves the small-op count on DVE.
        mvw = None
        xs_tiles = []  # last 8 (g, xs) tiles, two windows
        for n in range(WPC):
            # -- h = gelu(repeat(eng_proj) + pos_proj), to fp8 -------------
            # pre[e, ec, w] = epjT[e, ec, k(w)] + ppj[e, ec, w]; the repeat
            # is a stride-0 inner dim on the epjT read.  One fused 4D
            # gpsimd add + one fused ACT gelu per window (when b1 == 0).
            h_t = h_pool.tile([128, DC, W], FP8, tag="h", name="h")
            pre = pre_pool.tile([128, DC, W], BF16, tag="pre", name="pre")
            eng_bc = (epjT[:]
                      .rearrange("p (e t) -> p e t", e=DC)[:, :, n * K:(n + 1) * K]
                      .unsqueeze(3).broadcast_to([128, DC, K, REP]))
            if not has_b1:
                nc.gpsimd.tensor_tensor(
                    pre[:].rearrange("p e (k r) -> p e k r", r=REP),
                    eng_bc,
                    ppj[:].rearrange("p e (k r) -> p e k r", r=REP),
                    OP.add,
                )
                nc.scalar.activation(
                    h_t[:].rearrange("p e w -> p (e w)"),
                    pre[:].rearrange("p e w -> p (e w)"),
                    AF.Gelu,
                )
            else:
                for ec in range(DC):
                    ebc = epjT[:, ec * 128 + n * K: ec * 128 + (n + 1) * K]
                    ebc = ebc.unsqueeze(2).broadcast_to([128, K, REP])
                    nc.gpsimd.tensor_tensor(
                        pre[:, ec, :].rearrange("p (k r) -> p k r", r=REP),
                        ebc,
                        ppj[:, ec, :].rearrange("p (k r) -> p k r", r=REP),
                        OP.add,
                    )
                    nc.scalar.activation(
                        h_t[:, ec, :], pre[:, ec, :], AF.Gelu,
                        bias=b1c_sb[:, ec:ec + 1],
                    )

            # -- second matmul (fp8 DoubleRow) + residual, LN stats --------
            if n % 2 == 0:
                mvw = st_pool.tile([128, 16], FP32, tag="mvw", name="mvw")
                xs_tiles = []
            for g in range(4):
                px = x_pool.tile([128, D], FP32, tag="px", name="px")
                tsl = slice(g * 128, (g + 1) * 128)
                ssl = slice(n * W + g * 128, n * W + (g + 1) * 128)
                for half in range(2):
                    sl = slice(half * 512, half * 512 + 512)
                    for j in range(DC // 2):
                        nc.tensor.matmul(
                            px[:, sl],
                            lhsT=h_t[:, 2 * j:2 * j + 2, tsl],
                            rhs=w2p_sb[:, 2 * j:2 * j + 2, sl],
                            start=(j == 0),
                            stop=False,
                            perf_mode=DR,
                        )
                    nc.tensor.matmul(
                        px[:, sl],
                        lhsT=selt_sb[:, ssl],
                        rhs=eng_sb[:, sl],
                        start=False,
                        stop=False,
                    )
                    if has_b2:
                        nc.tensor.matmul(
                            px[:, sl],
                            lhsT=ones_sb[:],
                            rhs=b2_sb[:, sl],
                            start=False,
                            stop=False,
                        )
                    nc.tensor.matmul(
                        px[:, sl],
                        lhsT=id_sb[:],
                        rhs=pos_sb[g][:, sl],
                        start=False,
                        stop=True,
                    )
                # psum -> bf16 xs copy (GpSimd has no PSUM port: ACT/DVE only)
                xs = xs_pool.tile([128, D], BF16, tag="xs", name="xs")
                if g < 2:
                    nc.scalar.activation(xs[:], px[:], AF.Identity)
                else:
                    nc.vector.tensor_copy(xs[:], px[:])
                st = st_pool.tile([128, 12], FP32, tag="st", name="st")
                nc.vector.bn_stats(st[:, 0:6], xs[:, 0:512])
                nc.vector.bn_stats(st[:, 6:12], xs[:, 512:1024])
                nc.vector.bn_aggr(
                    mvw[:, (n % 2) * 8 + 2 * g:(n % 2) * 8 + 2 * g + 2],
                    st[:].rearrange("p (n s) -> p n s", s=3),
                )
                xs_tiles.append(xs)
            if n % 2 == 0:
                continue

            # -- rstd via 2 Newton iterations on DVE, [128, 8] per pair ----
            # PSUM holds 256*x, so var' = var_psum/2^16 ~ var_true; the
            # last Newton step folds in /256 so the output is unscaled.
            vw = nt_pool.tile([128, 8], FP32, tag="vw", name="vw")
            nc.vector.tensor_scalar(
                vw[:], mvw[:, 1::2], 1.0 / 65536.0, LN_EPS, OP.mult, OP.add)
            t0 = nt_pool.tile([128, 8], FP32, tag="t0", name="t0")
            nc.vector.tensor_scalar(t0[:], vw[:], 0.5, 0.5, OP.mult, OP.add)
            y = nt_pool.tile([128, 8], FP32, tag="y", name="y")
            nc.vector.reciprocal(y[:], t0[:])
            ys = None
            for it in range(NEWTON_ITERS):
                last = it == NEWTON_ITERS - 1
                y2 = nt_pool.tile([128, 8], FP32, tag="y2", name="y2")
                nc.vector.tensor_mul(y2[:], y[:], y[:])
                t = nt_pool.tile([128, 8], FP32, tag="t", name="t")
                nc.vector.tensor_mul(t[:], y2[:], vw[:])
                c = nt_pool.tile([128, 8], FP32, tag="c", name="c")
                s = 1.0 / SCALE if last else 1.0
                nc.vector.tensor_scalar(
                    c[:], t[:], -0.5 * s, 1.5 * s, OP.mult, OP.add)
                yn = nt_pool.tile([128, 8], FP32, tag="y", name="y")
                nc.vector.tensor_mul(yn[:], y[:], c[:])
                y = yn
            ys = y  # = rstd/256
            nm = nt_pool.tile([128, 8], FP32, tag="nm", name="nm")
            nc.vector.tensor_scalar_mul(nm[:], mvw[:, 0::2], -1.0)
            nmy = nt_pool.tile([128, 8], FP32, tag="nmy", name="nmy")
            nc.vector.tensor_mul(nmy[:], nm[:], ys[:])

            # -- normalize + store both windows of the pair ---------------
            for i, xs in enumerate(xs_tiles):
                m, g = divmod(i, 4)
                row0 = (n - 1 + m) * W + g * 128
                q = slice(i, i + 1)
                if not has_gb:
                    ot = out_pool.tile([128, D], FP32, tag="ot", name="ot")
                    if g < 2:
                        nc.scalar.activation(
                            ot[:], xs[:], AF.Identity,
                            bias=nmy[:, q], scale=ys[:, q],
                        )
                    elif g == 2:
                        nc.vector.tensor_scalar(
                            ot[:], xs[:], nm[:, q], ys[:, q], OP.add, OP.mult)
                    else:
                        nc.gpsimd.tensor_scalar(
                            ot[:], xs[:], nm[:, q], ys[:, q], OP.add, OP.mult)
                else:
                    xn = out_pool.tile([128, D], FP32, tag="xn", name="xn")
                    nc.gpsimd.tensor_scalar(
                        xn[:], xs[:], nm[:, q], ys[:, q], OP.add, OP.mult,
                    )
                    ot = out_pool.tile([128, D], FP32, tag="ot", name="ot")
                    nc.vector.scalar_tensor_tensor(
                        ot[:], xn[:], 1.0, gam_sb[:], OP.mult, OP.mult
                    )
                    nc.vector.tensor_add(ot[:], ot[:], bet_sb[:])
                nc.sync.dma_start(out_d[row0:row0 + 128, :], ot[:])

    nc.compile()
    return nc


def _get_program(has_b1, has_b2, has_gb):
    key = (has_b1, has_b2, has_gb)
    if key not in _PROGRAM_CACHE:
        _PROGRAM_CACHE[key] = _build_program(*key)
    return _PROGRAM_CACHE[key]


def _make_in_maps(engrams, pos_emb, w1, b1, w2, b2, gamma, beta,
                  has_b1, has_b2, has_gb):
    bf16 = ml_dtypes.bfloat16
    fp8 = ml_dtypes.float8_e4m3fn
    eng_flat = np.asarray(engrams, np.float32).reshape(B * NW, K, D)
    pos = np.asarray(pos_emb, np.float32).reshape(W, D)
    w1t = np.asarray(w1, np.float32).T
    # w2p[ki, ec*1024 + n] = w2.T[ec*128 + ki, n] * 256
    w2t_s = np.asarray(w2, np.float32).T * SCALE
    w2p = np.ascontiguousarray(
        w2t_s.reshape(DC, 128, D).transpose(1, 0, 2).reshape(128, DC * D)
    ).astype(fp8)
    post = np.ascontiguousarray(pos * SCALE).astype(bf16)
    posT = np.ascontiguousarray(pos.T).astype(bf16)       # [D, W], unscaled
    selt = np.kron(np.eye(128, dtype=np.float32),
                   np.ones((1, REP), np.float32)).astype(bf16)  # [128, 4096]
    idn = np.eye(128, dtype=np.float32).astype(bf16)

    shared = {"w1t": np.ascontiguousarray(w1t).astype(bf16), "w2p": w2p,
              "post": post, "selt": selt, "idn": idn}
    if has_b1:
        shared["b1c"] = np.ascontiguousarray(
            np.asarray(b1, np.float32).reshape(DC, 128).T)
    if has_b2:
        shared["b2r"] = (np.asarray(b2, np.float32).reshape(1, D)
                         * SCALE).astype(bf16)
    if has_gb:
        shared["gam"] = np.ascontiguousarray(
            np.asarray(gamma, np.float32).reshape(1, D))
        shared["bet"] = np.ascontiguousarray(
            np.asarray(beta, np.float32).reshape(1, D))

    in_maps = []
    for c in range(N_CORES):
        eng_c = eng_flat[c * WPC:(c + 1) * WPC]           # [WPC, K, D]
        engT = eng_c.reshape(WPC * K, D).T                # [D, 128], unscaled
        ebt = np.concatenate([engT, posT.astype(np.float32)], axis=1)
        ebt = np.ascontiguousarray(ebt).astype(bf16)      # [D, 640]
        eng256 = np.ascontiguousarray(
            eng_c.reshape(WPC * K, D) * SCALE).astype(bf16)
        in_maps.append({"ebt": ebt, "eng": eng256, **shared})
    return in_maps


def kernel(engrams, pos_emb, w1, b1, w2, b2, gamma, beta):
    has_b1 = bool(np.any(np.asarray(b1) != 0))
    has_b2 = bool(np.any(np.asarray(b2) != 0))
    has_gb = bool(np.any(np.asarray(gamma) != 1) or np.any(np.asarray(beta) != 0))

    nc = _get_program(has_b1, has_b2, has_gb)
    in_maps = _make_in_maps(engrams, pos_emb, w1, b1, w2, b2, gamma, beta,
                            has_b1, has_b2, has_gb)
    res = run_bass_kernel_spmd(nc, in_maps, list(range(N_CORES)))
    full = np.concatenate([res.results[c]["out"] for c in range(N_CORES)], axis=0)
    return np.ascontiguousarray(
        full.reshape(B, NW, W, D).astype(np.float32, copy=False))


# revision 20
# speedup vs baseline: 1.1133x; 1.1133x over previous
"""TRN2 Bass/Tile kernel for nn_EngramUpsampler (dense_mlp), v2.

Reference computation:
    x   = repeat_interleave(engrams, 32, axis=2) + pos_emb   # (B, NW, 512, 1024)
    h   = gelu(x @ w1.T + b1, exact)
    x   = x + h @ w2.T + b2
    out = LayerNorm(x) * gamma + beta

Distribution: data-parallel over the B*NW = 64 (batch, window) slices,
8 windows per NeuronCore; weights/pos_emb replicated.

v2 design (vs baseline 206.7us):
  * First matmul factors through the repeat (21x FLOP cut):
        x @ w1.T = repeat(eng @ w1.T) + pos @ w1.T
    Both projections are computed directly in [e, token] orientation from
    HOST-transposed inputs (ebT = [engT | posT]), eliminating all 48 PE
    transposes and their DVE copies of the baseline.
  * h = gelu(...) is emitted by ScalarE directly as fp8e4 into DoubleRow
    pair layout [128, 8ec, 512w]; the second matmul h @ w2.T runs as
    fp8 DoubleRow (2 contraction chunks per instruction, ~1.4-1.8x PE).
    w2.T is pre-scaled by 256 (host) to dodge fp8 subnormals; the residual
    matmul inputs (eng, pos) are also uploaded x256 in bf16, so PSUM
    uniformly holds 256*x.  LayerNorm is scale-invariant: no descale pass,
    only the rstd constants fold in 2^-8 and eps uses var' = var*2^-16.
  * Residual x0 = repeat(eng) + pos joins the same PSUM accumulation via
    R/I matmuls (TensorE), b2 via a ones-row matmul when nonzero.
  * LayerNorm: bn_stats/bn_aggr on DVE; rsqrt via 3 Newton iterations on
    DVE (initial guess 1/(0.5+0.5v) is ~1% accurate for v near 1);
    normalization via ACT Identity(x*rstd - mu*rstd) on 2 tiles and
    GpSimd tensor_scalar on 2, balancing engine load under the PE time.
"""

import numpy as np
import ml_dtypes

import concourse.bass as bass
import concourse.tile as tile
from concourse import bacc, mybir
from concourse.bass_utils import run_bass_kernel_spmd

FP32 = mybir.dt.float32
BF16 = mybir.dt.bfloat16
FP8 = mybir.dt.float8e4
AF = mybir.ActivationFunctionType
OP = mybir.AluOpType
DR = mybir.MatmulPerfMode.DoubleRow

B, NW, K, D, W = 4, 16, 16, 1024, 512
REP = W // K          # 32
N_CORES = 8
WPC = (B * NW) // N_CORES   # 8 windows per core
TOK = WPC * W               # 4096 tokens per core
DC = D // 128               # 8 chunks of the contraction dim
LN_EPS = 1e-5
NEWTON_ITERS = 2
SCALE = 256.0               # x256 on w2/eng/pos; LN folds it back out

_PROGRAM_CACHE: dict = {}


def _build_program(has_b1: bool, has_b2: bool, has_gb: bool):
    """Emit + compile the per-core SPMD program. Same program runs on all 8
    cores; only the DRAM input contents differ."""
    from contextlib import ExitStack

    nc = bacc.Bacc("TRN2", target_bir_lowering=False, debug=False)

    # ebT[d, 0:128] = eng.T chunk, ebT[d, 128:640] = pos.T chunk (bf16)
    ebt_d = nc.dram_tensor("ebt", [D, 640], BF16, kind="ExternalInput").ap()
    w1t_d = nc.dram_tensor("w1t", [D, D], BF16, kind="ExternalInput").ap()
    # w2p[ki, ec, n] = w2.T[ec*128+ki, n] * 256 in fp8e4 (DoubleRow pairs)
    w2p_d = nc.dram_tensor("w2p", [128, DC * D], FP8, kind="ExternalInput").ap()
    # selt[j, c] = 1 iff c//32 == j: window n / token-tile g residual selector
    # is the slice selt[:, 512n + 128g : ...], picking engram row 16n + t//32.
    selt_d = nc.dram_tensor("selt", [128, TOK], BF16, kind="ExternalInput").ap()
    # eng256[16n + k, :] = eng[n, k, :] * 256 -- all 8 windows in one tile
    eng_d = nc.dram_tensor("eng", [128, D], BF16, kind="ExternalInput").ap()
    post_d = nc.dram_tensor("post", [W, D], BF16, kind="ExternalInput").ap()  # x256
    idn_d = nc.dram_tensor("idn", [128, 128], BF16, kind="ExternalInput").ap()
    if has_b1:
        b1c_d = nc.dram_tensor("b1c", [128, DC], FP32, kind="ExternalInput").ap()
    if has_b2:
        b2r_d = nc.dram_tensor("b2r", [1, D], BF16, kind="ExternalInput").ap()  # x256
    if has_gb:
        gam_d = nc.dram_tensor("gam", [1, D], FP32, kind="ExternalInput").ap()
        bet_d = nc.dram_tensor("bet", [1, D], FP32, kind="ExternalInput").ap()
    out_d = nc.dram_tensor("out", [TOK, D], FP32, kind="ExternalOutput").ap()

    with tile.TileContext(nc) as tc, ExitStack() as ctx:
        consts = ctx.enter_context(tc.tile_pool(name="consts", bufs=1))

        def ctile(shape, dtype, tag):
            return consts.tile(shape, dtype, tag=tag, name=tag)

        # ---- persistent SBUF tensors -------------------------------------
        w1t_sb = [ctile([128, D], BF16, f"w1t{i}") for i in range(DC)]
        ebt_sb = [ctile([128, 640], BF16, f"ebt{i}") for i in range(DC)]
        w2p_sb = ctile([128, DC, D], FP8, "w2p")
        selt_sb = ctile([128, TOK], BF16, "selt")
        id_sb = ctile([128, 128], BF16, "id_sb")
        eng_sb = ctile([128, D], BF16, "eng")
        pos_sb = [ctile([128, D], BF16, f"pos{i}") for i in range(4)]
        epjT = ctile([128, D], BF16, "epjT")               # [e, tok] per ec block
        ppj = ctile([128, DC, W], BF16, "ppj")             # [e, ec, w]
        if has_b1:
            b1c_sb = ctile([128, DC], FP32, "b1c")
        if has_b2:
            ones_sb = ctile([1, 128], BF16, "ones")
            b2_sb = ctile([1, D], BF16, "b2")
        if has_gb:
            gam_row = ctile([1, D], FP32, "gamr")
            bet_row = ctile([1, D], FP32, "betr")
            gam_sb = ctile([128, D], FP32, "gam")
            bet_sb = ctile([128, D], FP32, "bet")

        # ---- loads. Head is DMA-transfer-bound: split the setup-critical
        # w1t+ebt across BOTH hwdge queues (sync, scalar), and push the
        # main-loop-only tensors to the gpsimd SWDGE queue. ---------------
        for i in range(DC):
            if i < DC // 2:
                nc.sync.dma_start(w1t_sb[i][:], w1t_d[i * 128:(i + 1) * 128, :])
                nc.scalar.dma_start(ebt_sb[i][:], ebt_d[i * 128:(i + 1) * 128, :])
            else:
                nc.scalar.dma_start(w1t_sb[i][:], w1t_d[i * 128:(i + 1) * 128, :])
                nc.sync.dma_start(ebt_sb[i][:], ebt_d[i * 128:(i + 1) * 128, :])
        nc.sync.dma_start(
            w2p_sb[:].rearrange("p a b -> p (a b)"), w2p_d[:])
        nc.sync.dma_start(id_sb[:], idn_d[:])
        nc.gpsimd.dma_start(eng_sb[:], eng_d[:])
        for i in range(4):
            nc.gpsimd.dma_start(pos_sb[i][:], post_d[i * 128:(i + 1) * 128, :])
        nc.gpsimd.dma_start(selt_sb[:], selt_d[:])
        if has_b1:
            nc.sync.dma_start(b1c_sb[:], b1c_d[:])
        if has_b2:
            nc.gpsimd.memset(ones_sb[:], 1.0)
            nc.sync.dma_start(b2_sb[:], b2r_d[:])
        if has_gb:
            nc.sync.dma_start(gam_row[:], gam_d[:])
            nc.sync.dma_start(bet_row[:], bet_d[:])
            nc.gpsimd.partition_broadcast(gam_sb[:], gam_row[:])
            nc.gpsimd.partition_broadcast(bet_sb[:], bet_row[:])

        # ---- setup: factored first matmul in [e, token] orientation ------
        # epjT[e128, t] = sum_d w1t[d, e] engT[d, t];  ppj[e128, w] likewise
        with tc.tile_pool(name="spsum", bufs=2, space="PSUM") as spsum:
            for ec in range(DC):
                pje = spsum.tile([128, 128], FP32, tag="pje", name="pje")
                pjp = spsum.tile([128, W], FP32, tag="pjp", name="pjp")
                esl = slice(ec * 128, (ec + 1) * 128)
                for dc in range(DC):
                    nc.tensor.matmul(
                        pje[:],
                        lhsT=w1t_sb[dc][:, esl],
                        rhs=ebt_sb[dc][:, 0:128],
                        start=(dc == 0),
                        stop=(dc == DC - 1),
                    )
                for dc in range(DC):
                    nc.tensor.matmul(
                        pjp[:],
                        lhsT=w1t_sb[dc][:, esl],
                        rhs=ebt_sb[dc][:, 128:640],
                        start=(dc == 0),
                        stop=(dc == DC - 1),
                    )
                nc.vector.tensor_copy(epjT[:, esl], pje[:])
                nc.vector.tensor_copy(ppj[:, ec, :], pjp[:])

        # ---- main loop ---------------------------------------------------
        pre_pool = ctx.enter_context(tc.tile_pool(name="pre", bufs=3))
        h_pool = ctx.enter_context(tc.tile_pool(name="h", bufs=3))
        x_pool = ctx.enter_context(tc.tile_pool(name="xps", bufs=3, space="PSUM"))
        xs_pool = ctx.enter_context(tc.tile_pool(name="xs", bufs=10))
        st_pool = ctx.enter_context(tc.tile_pool(name="st", bufs=4))
        nt_pool = ctx.enter_context(tc.tile_pool(name="nt", bufs=2))
        out_pool = ctx.enter_context(tc.tile_pool(name="ot", bufs=4))

        # LN stats/normalize are batched over GROUPS of windows: one Newton
        # chain of [128, 4*len] per group amortizes the small-op DVE cost.
        # The last two windows run solo so the output drain starts early.
        GROUPS = [(0, 1), (2, 3), (4, 5), (6,), (7,)]
        GRP_OF = {n: gi for gi, grp in enumerate(GROUPS) for n in grp}
        mvw = None
        xs_tiles = []
        for n in range(WPC):
            # -- h = gelu(repeat(eng_proj) + pos_proj), to fp8 -------------
            # pre[e, ec, w] = epjT[e, ec, k(w)] + ppj[e, ec, w]; the repeat
            # is a stride-0 inner dim on the epjT read.  Work in PAIRS of
            # e-chunks: fused enough to amortize overhead, fine-grained
            # enough that the PE never waits a whole window for h.
            h_t = h_pool.tile([128, DC, W], FP8, tag="h", name="h")
            pre = pre_pool.tile([128, DC, W], BF16, tag="pre", name="pre")
            epjT_v = epjT[:].rearrange("p (e t) -> p e t", e=DC)
            for j in range(DC // 2):
                js = slice(2 * j, 2 * j + 2)
                ebc = (epjT_v[:, js, n * K:(n + 1) * K]
                       .unsqueeze(3).broadcast_to([128, 2, K, REP]))
                nc.gpsimd.tensor_tensor(
                    pre[:, js, :].rearrange("p e (k r) -> p e k r", r=REP),
                    ebc,
                    ppj[:, js, :].rearrange("p e (k r) -> p e k r", r=REP),
                    OP.add,
                )
                if not has_b1:
                    nc.scalar.activation(
                        h_t[:, js, :].rearrange("p e w -> p (e w)"),
                        pre[:, js, :].rearrange("p e w -> p (e w)"),
                        AF.Gelu,
                    )
                else:
                    for ec in (2 * j, 2 * j + 1):
                        nc.scalar.activation(
                            h_t[:, ec, :], pre[:, ec, :], AF.Gelu,
                            bias=b1c_sb[:, ec:ec + 1],
                        )

            # -- second matmul (fp8 DoubleRow) + residual, LN stats --------
            grp = GROUPS[GRP_OF[n]]
            m = n - grp[0]
            if m == 0:
                mvw = st_pool.tile([128, 16], FP32, tag="mvw", name="mvw")
                xs_tiles = []
            for g in range(4):
                px = x_pool.tile([128, D], FP32, tag="px", name="px")
                tsl = slice(g * 128, (g + 1) * 128)
                ssl = slice(n * W + g * 128, n * W + (g + 1) * 128)
                for half in range(2):
                    sl = slice(half * 512, half * 512 + 512)
                    for j in range(DC // 2):
                        nc.tensor.matmul(
                            px[:, sl],
                            lhsT=h_t[:, 2 * j:2 * j + 2, tsl],
                            rhs=w2p_sb[:, 2 * j:2 * j + 2, sl],
                            start=(j == 0),
                            stop=False,
                            perf_mode=DR,
                        )
                    nc.tensor.matmul(
                        px[:, sl],
                        lhsT=selt_sb[:, ssl],
                        rhs=eng_sb[:, sl],
                        start=False,
                        stop=False,
                    )
                    if has_b2:
                        nc.tensor.matmul(
                            px[:, sl],
                            lhsT=ones_sb[:],
                            rhs=b2_sb[:, sl],
                            start=False,
                            stop=False,
                        )
                    nc.tensor.matmul(
                        px[:, sl],
                        lhsT=id_sb[:],
                        rhs=pos_sb[g][:, sl],
                        start=False,
                        stop=True,
                    )
                # psum -> bf16 xs copy (GpSimd has no PSUM port: ACT/DVE only)
                xs = xs_pool.tile([128, D], BF16, tag="xs", name="xs")
                if g < 3:
                    nc.scalar.activation(xs[:], px[:], AF.Identity)
                else:
                    nc.vector.tensor_copy(xs[:], px[:])
                st = st_pool.tile([128, 12], FP32, tag="st", name="st")
                nc.vector.bn_stats(st[:, 0:6], xs[:, 0:512])
                nc.vector.bn_stats(st[:, 6:12], xs[:, 512:1024])
                # mean -> col m*4+g, var -> col 8+m*4+g: keeps the mean and
                # var blocks contiguous for the Newton chain reads.
                c0 = m * 4 + g
                nc.vector.bn_aggr(
                    mvw[:, c0:c0 + 9:8],
                    st[:].rearrange("p (n s) -> p n s", s=3),
                )
                xs_tiles.append(xs)
            if n != grp[-1]:
                continue

            # -- rstd via 2 Newton iterations on DVE, [128, 4L] per group --
            # PSUM holds 256*x, so var' = var_psum/2^16 ~ var_true; the
            # last Newton step folds in /256 so the output is unscaled.
            L4 = 4 * len(grp)
            vw = nt_pool.tile([128, L4], FP32, tag="vw", name="vw")
            nc.vector.tensor_scalar(
                vw[:], mvw[:, 8:8 + L4], 1.0 / 65536.0, LN_EPS,
                OP.mult, OP.add)
            t0 = nt_pool.tile([128, L4], FP32, tag="t0", name="t0")
            nc.vector.tensor_scalar(t0[:], vw[:], 0.5, 0.5, OP.mult, OP.add)
            y = nt_pool.tile([128, L4], FP32, tag="y", name="y")
            nc.vector.reciprocal(y[:], t0[:])
            for it in range(NEWTON_ITERS):
                last = it == NEWTON_ITERS - 1
                y2 = nt_pool.tile([128, L4], FP32, tag="y2", name="y2")
                nc.vector.tensor_mul(y2[:], y[:], y[:])
                t = nt_pool.tile([128, L4], FP32, tag="t", name="t")
                nc.vector.tensor_mul(t[:], y2[:], vw[:])
                c = nt_pool.tile([128, L4], FP32, tag="c", name="c")
                s = 1.0 / SCALE if last else 1.0
                nc.vector.tensor_scalar(
                    c[:], t[:], -0.5 * s, 1.5 * s, OP.mult, OP.add)
                yn = nt_pool.tile([128, L4], FP32, tag="y", name="y")
                nc.vector.tensor_mul(yn[:], y[:], c[:])
                y = yn
            ys = y  # = rstd/256
            # nmy = (-mean) * rstd/256, all-contiguous reads
            nmy = nt_pool.tile([128, L4], FP32, tag="nmy", name="nmy")
            nc.vector.scalar_tensor_tensor(
                nmy[:], mvw[:, 0:L4], -1.0, ys[:], OP.mult, OP.mult)

            # -- normalize (out = xs*ys + nmy) + store --------------------
            for i, xs in enumerate(xs_tiles):
                mm_, g = divmod(i, 4)
                row0 = (grp[0] + mm_) * W + g * 128
                q = slice(i, i + 1)
                if not has_gb:
                    ot = out_pool.tile([128, D], FP32, tag="ot", name="ot")
                    if g == 0:
                        nc.vector.tensor_scalar(
                            ot[:], xs[:], ys[:, q], nmy[:, q],
                            OP.mult, OP.add)
                    else:
                        nc.gpsimd.tensor_scalar(
                            ot[:], xs[:], ys[:, q], nmy[:, q],
                            OP.mult, OP.add)
                else:
                    xn = out_pool.tile([128, D], FP32, tag="xn", name="xn")
                    nc.gpsimd.tensor_scalar(
                        xn[:], xs[:], ys[:, q], nmy[:, q], OP.mult, OP.add,
                    )
                    ot = out_pool.tile([128, D], FP32, tag="ot", name="ot")
                    nc.vector.scalar_tensor_tensor(
                        ot[:], xn[:], 1.0, gam_sb[:], OP.mult, OP.mult
                    )
                    nc.vector.tensor_add(ot[:], ot[:], bet_sb[:])
                if i % 2 == 0:
                    nc.sync.dma_start(out_d[row0:row0 + 128, :], ot[:])
                else:
                    nc.scalar.dma_start(out_d[row0:row0 + 128, :], ot[:])

    nc.compile()
    return nc


def _get_program(has_b1, has_b2, has_gb):
    key = (has_b1, has_b2, has_gb)
    if key not in _PROGRAM_CACHE:
        _PROGRAM_CACHE[key] = _build_program(*key)
    return _PROGRAM_CACHE[key]


def _make_in_maps(engrams, pos_emb, w1, b1, w2, b2, gamma, beta,
                  has_b1, has_b2, has_gb):
    bf16 = ml_dtypes.bfloat16
    fp8 = ml_dtypes.float8_e4m3fn
    eng_flat = np.asarray(engrams, np.float32).reshape(B * NW, K, D)
    pos = np.asarray(pos_emb, np.float32).reshape(W, D)
    w1t = np.asarray(w1, np.float32).T
    # w2p[ki, ec*1024 + n] = w2.T[ec*128 + ki, n] * 256
    w2t_s = np.asarray(w2, np.float32).T * SCALE
    w2p = np.ascontiguousarray(
        w2t_s.reshape(DC, 128, D).transpose(1, 0, 2).reshape(128, DC * D)
    ).astype(fp8)
    post = np.ascontiguousarray(pos * SCALE).astype(bf16)
    posT = np.ascontiguousarray(pos.T).astype(bf16)       # [D, W], unscaled
    selt = np.kron(np.eye(128, dtype=np.float32),
                   np.ones((1, REP), np.float32)).astype(bf16)  # [128, 4096]
    idn = np.eye(128, dtype=np.float32).astype(bf16)

    shared = {"w1t": np.ascontiguousarray(w1t).astype(bf16), "w2p": w2p,
              "post": post, "selt": selt, "idn": idn}
    if has_b1:
        shared["b1c"] = np.ascontiguousarray(
            np.asarray(b1, np.float32).reshape(DC, 128).T)
    if has_b2:
        shared["b2r"] = (np.asarray(b2, np.float32).reshape(1, D)
                         * SCALE).astype(bf16)
    if has_gb:
        shared["gam"] = np.ascontiguousarray(
            np.asarray(gamma, np.float32).reshape(1, D))
        shared["bet"] = np.ascontiguousarray(
            np.asarray(beta, np.float32).reshape(1, D))

    in_maps = []
    for c in range(N_CORES):
        eng_c = eng_flat[c * WPC:(c + 1) * WPC]           # [WPC, K, D]
        engT = eng_c.reshape(WPC * K, D).T                # [D, 128], unscaled
        ebt = np.concatenate([engT, posT.astype(np.float32)], axis=1)
        ebt = np.ascontiguousarray(ebt).astype(bf16)      # [D, 640]
        eng256 = np.ascontiguousarray(
            eng_c.reshape(WPC * K, D) * SCALE).astype(bf16)
        in_maps.append({"ebt": ebt, "eng": eng256, **shared})
    return in_maps


def kernel(engrams, pos_emb, w1, b1, w2, b2, gamma, beta):
    has_b1 = bool(np.any(np.asarray(b1) != 0))
    has_b2 = bool(np.any(np.asarray(b2) != 0))
    has_gb = bool(np.any(np.asarray(gamma) != 1) or np.any(np.asarray(beta) != 0))

    nc = _get_program(has_b1, has_b2, has_gb)
    in_maps = _make_in_maps(engrams, pos_emb, w1, b1, w2, b2, gamma, beta,
                            has_b1, has_b2, has_gb)
    res = run_bass_kernel_spmd(nc, in_maps, list(range(N_CORES)))
    full = np.concatenate([res.results[c]["out"] for c in range(N_CORES)], axis=0)
    return np.ascontiguousarray(
        full.reshape(B, NW, W, D).astype(np.float32, copy=False))
